# revision 10
# baseline (speedup 1.0000x reference)
"""Trainium2 Bass kernel for nn_Encoder_73478300500500.

GRU-based gumbel sampler:
  h0 = gelu(x @ fc_w.T); 20 steps of GRU + proj + gumbel-argmax one-hot
  feedback; outputs (message [B,T,9] one-hot-ish fp32, lengths [B] int32).

Strategy (8 cores, pure batch data-parallel, 4096 batch rows/core):
  * All recurrent state kept transposed: hT [H=1024 partitions(8 chunks), batch free].
    Main matmul per step: ghT = w_hh @ hT done as 24 m-chunks x 8 k-chunks of
    [128,128] fp32 stationary x [128, NB] moving -> PSUM, so h_new lands in the
    layout the next step consumes (no per-step transposes of h).
  * The fed-back token is one-hot -> tok @ w_ih.T == column gather of a
    [9, 3H] table == a K=9 matmul accumulated into the same PSUM groups.
    Biases b_ih+b_hh are folded into that table (token rows sum to 1).
  * r/z gates share one [128, 2*NB] PSUM (single sigmoid per j-chunk);
    n-gate keeps h_n and i_n in separate PSUMs (n = tanh(i_n + r*h_n)).
  * proj -> logitsT [9, NB] psum; gumbel (+proj_b, host-pretransposed) added;
    PE-transpose 128-col slices to [128, 9]; free-dim reduce_max + is_equal
    against the per-partition max gives the one-hot mask == message output;
    mask transposed back to [9, NB] for the next step's token matmuls.
  * lengths: EOS column of mask accumulated per step into eos[128, NCH, T];
    post-loop: lengths = 21 - max(max_t(eos_t * (T - t)), 1).
  * fc phase per pair: fc_w prescaled by 0.5 host-side; gelu computed as
    u'*(1+erf(sqrt(2)*u')) with u' = 0.5*x@fc_w.T (stays in the
    sigmoid/tanh/erf ACT table set).
  * 2 blocks of NB=512 interleaved per "pair" to hide gate/sampling latency
    between steps; 4 sequential pairs per core.
"""

import os
import sys

import numpy as np

for _p in ("/opt/trn_rl_repo", os.path.expanduser("~/.axon_site/_ro/trn_rl_repo")):
    if os.path.isdir(_p) and _p not in sys.path:
        sys.path.insert(0, _p)

os.environ.setdefault("MYCRO_LOCAL_CACHE", "1")

import concourse.bacc as bacc  # noqa: E402
import concourse.bass as bass  # noqa: E402
import concourse.mybir as mybir  # noqa: E402
import concourse.tile as tile  # noqa: E402

F32 = mybir.dt.float32
AF = mybir.ActivationFunctionType
ALU = mybir.AluOpType

B_FULL, H, V, T_FULL = 32768, 1024, 9, 20
EOS = 8
NCORES = 8
BL_FULL = B_FULL // NCORES  # 4096
KC = H // 128               # 8 k-chunks of hidden
KF = 7                      # 784 -> 7 chunks of 128 (padded to 896)
SQRT2 = 1.4142135623730951


def build_nc(BL=BL_FULL, T=T_FULL, NB=512):
    """Build the single-core SPMD Bass program (identical across cores)."""
    assert BL % (2 * NB) == 0 and NB % 128 == 0 and NB <= 512
    PAIRS = BL // (2 * NB)
    NCH = NB // 128

    nc = bacc.Bacc("TRN2", target_bir_lowering=False, debug=False)

    xT_d = nc.dram_tensor("xT", [KF, 128, BL], F32, kind="ExternalInput")
    whh_d = nc.dram_tensor("whh", [128, KC, 3 * H], F32, kind="ExternalInput")
    atab_d = nc.dram_tensor("atab", [V, 4 * H], F32, kind="ExternalInput")
    projT_d = nc.dram_tensor("projT", [128, KC, V], F32, kind="ExternalInput")
    fcw_d = nc.dram_tensor("fcw", [KF, 128, H], F32, kind="ExternalInput")
    gt_d = nc.dram_tensor("gt", [T, V, BL], F32, kind="ExternalInput")
    wt_d = nc.dram_tensor("wt", [128, T], F32, kind="ExternalInput")
    ident_d = nc.dram_tensor("ident", [128, 128], F32, kind="ExternalInput")

    msg_d = nc.dram_tensor("msg", [BL, T, V], F32, kind="ExternalOutput")
    lenf_d = nc.dram_tensor("lenf", [BL], F32, kind="ExternalOutput")

    # message viewed as [p, chunk, t, v] so a [128, NCH, 9] sbuf tile DMAs out
    # with matching element order
    msg_v = msg_d.rearrange("(c p) t v -> p c t v", p=128)

    with tile.TileContext(nc) as tc:
        with tc.tile_pool(name="persist", bufs=1) as pp:
            whh_sb = pp.tile([128, KC, 3 * H], F32, name="whh_sb")
            nc.sync.dma_start(out=whh_sb[:], in_=whh_d[:])
            atab_sb = pp.tile([V, 4 * H], F32, name="atab_sb")
            nc.sync.dma_start(out=atab_sb[:], in_=atab_d[:])
            projT_sb = pp.tile([128, KC, V], F32, name="projT_sb")
            nc.sync.dma_start(out=projT_sb[:], in_=projT_d[:])
            ident_sb = pp.tile([128, 128], F32, name="ident_sb")
            nc.sync.dma_start(out=ident_sb[:], in_=ident_d[:])
            wt_sb = pp.tile([128, T], F32, name="wt_sb")
            nc.sync.dma_start(out=wt_sb[:], in_=wt_d[:])

            for pair in range(PAIRS):
                _emit_pair(nc, tc, pair, BL, T, NB, NCH,
                           whh_sb, atab_sb, projT_sb, ident_sb, wt_sb,
                           xT_d, fcw_d, gt_d, msg_v, lenf_d)
    nc.compile()
    return nc


def _hbuf_idx(T):
    """3-buffer rotation: A(t): a[t]->a[t+1]; B(t): b[t]->b[t+1], with
    a[t+1] = 3-a[t]-b[t], b[t+1] = a[t]. Guarantees the buffer being written
    is dead for both blocks (reads of it finished >= a full block-step ago)."""
    a, b = [0], [1]
    for t in range(T):
        a.append(3 - a[-1] - b[-1])
        b.append(a[-2])
    return a, b


def _emit_pair(nc, tc, pair, BL, T, NB, NCH,
               whh_sb, atab_sb, projT_sb, ident_sb, wt_sb,
               xT_d, fcw_d, gt_d, msg_v, lenf_d):
    with tc.tile_pool(name=f"hp{pair}", bufs=1) as hp:
        hbufs = [hp.tile([128, KC, NB], F32, name=f"h_{pair}_{i}", bufs=1)
                 for i in range(3)]
        ha, hb = _hbuf_idx(T)
        eos = [hp.tile([128, NCH, T], F32, name=f"eos_{pair}_{b}", bufs=1)
               for b in range(2)]
        tok = [hp.tile([V, NB], F32, name=f"tok_{pair}_{b}", bufs=1)
               for b in range(2)]

        # ---- fc phase: h0 = gelu(x @ fc_w.T), via prescaled-by-0.5 weights
        with tc.tile_pool(name=f"fc{pair}", bufs=1) as fcp, \
             tc.tile_pool(name=f"fcps{pair}", bufs=1, space="PSUM") as fpp:
            for b in range(2):
                n0 = pair * 2 * NB + b * NB
                fps = [fpp.tile([128, NB], F32, tag="fcps", bufs=8,
                                name=f"fps_{pair}_{b}_{m}") for m in range(KC)]
                for k in range(KF):
                    fcw_t = fcp.tile([128, H], F32, tag="fcw", bufs=2,
                                     name=f"fcw_t_{pair}_{b}_{k}")
                    nc.sync.dma_start(out=fcw_t[:], in_=fcw_d[k])
                    xt_t = fcp.tile([128, NB], F32, tag="xt", bufs=2,
                                    name=f"xt_t_{pair}_{b}_{k}")
                    nc.sync.dma_start(out=xt_t[:], in_=xT_d[k, :, n0:n0 + NB])
                    for m in range(KC):
                        nc.tensor.matmul(
                            fps[m][:], fcw_t[:, m * 128:(m + 1) * 128], xt_t[:],
                            start=(k == 0), stop=(k == KF - 1))
                h0 = hbufs[ha[0] if b == 0 else hb[0]]
                for m in range(KC):
                    e_t = fcp.tile([128, NB], F32, tag="erf", bufs=2,
                                   name=f"erf_{pair}_{b}_{m}")
                    nc.scalar.activation(e_t[:], fps[m][:], AF.Erf, scale=SQRT2)
                    # h0 = u' * (1 + erf(sqrt2*u')), u' = 0.5*x@fcwT
                    nc.vector.scalar_tensor_tensor(
                        h0[:, m, :], e_t[:], 1.0, fps[m][:],
                        op0=ALU.add, op1=ALU.mult)
                nc.vector.memset(tok[b][:, :], 0.0)
                nc.vector.memset(tok[b][0:1, :], 1.0)

        # ---- recurrence
        with tc.tile_pool(name=f"rec{pair}", bufs=1) as rp, \
             tc.tile_pool(name=f"rps{pair}", bufs=1, space="PSUM") as pps:
            for t in range(T):
                for b in range(2):
                    idx = (ha, hb)[b]
                    _emit_step(nc, tc, pair, b, t, BL, T, NB, NCH,
                               whh_sb, atab_sb, projT_sb, ident_sb,
                               hbufs[idx[t]], hbufs[idx[t + 1]],
                               eos[b], tok[b], gt_d, msg_v, rp, pps)
            # ---- lengths
            for b in range(2):
                for c in range(NCH):
                    cg = (pair * 2 * NB + b * NB) // 128 + c
                    ltmp = rp.tile([128, T], F32, tag="ltmp", bufs=2,
                                   name=f"ltmp_{pair}_{b}_{c}")
                    nc.vector.tensor_tensor(ltmp[:], eos[b][:, c, :],
                                            wt_sb[:, 0:T], op=ALU.mult)
                    lmx = rp.tile([128, 1], F32, tag="lmx", bufs=2,
                                  name=f"lmx_{pair}_{b}_{c}")
                    nc.vector.tensor_reduce(lmx[:], ltmp[:],
                                            axis=mybir.AxisListType.X,
                                            op=ALU.max)
                    lfin = rp.tile([128, 1], F32, tag="lfin", bufs=2,
                                   name=f"lfin_{pair}_{b}_{c}")
                    nc.vector.tensor_scalar(lfin[:], lmx[:], 1.0, None,
                                            op0=ALU.max)
                    nc.vector.tensor_scalar(lfin[:], lfin[:], -1.0,
                                            float(T + 1),
                                            op0=ALU.mult, op1=ALU.add)
                    nc.sync.dma_start(out=lenf_d[cg * 128:(cg + 1) * 128],
                                      in_=lfin[:])


def _emit_step(nc, tc, pair, b, t, BL, T, NB, NCH,
               whh_sb, atab_sb, projT_sb, ident_sb,
               hc, hx, eos, tok, gt_d, msg_v, rp, pps):
    """One GRU step for one block: reads hc (h_t), writes hx (h_{t+1})."""
    n0 = pair * 2 * NB + b * NB
    cg0 = n0 // 128

    # prefetch gumbel for this (block, step)
    gtt = rp.tile([V, NB], F32, tag=f"gt{b}", bufs=2, name=f"gt_{pair}_{b}_{t}")
    nc.sync.dma_start(out=gtt[:], in_=gt_d[t, :, n0:n0 + NB])

    for j in range(KC):
        rz = pps.tile([128, 2 * NB], F32, tag="rz", bufs=2,
                      name=f"rz_{pair}_{b}_{t}_{j}")
        hn = pps.tile([128, NB], F32, tag="hn", bufs=2,
                      name=f"hn_{pair}_{b}_{t}_{j}")
        inp = pps.tile([128, NB], F32, tag="misc", bufs=2,
                       name=f"inp_{pair}_{b}_{t}_{j}")
        for k in range(KC):
            nc.tensor.matmul(rz[:, 0:NB],
                             whh_sb[:, k, j * 128:(j + 1) * 128],
                             hc[:, k, :], start=(k == 0), stop=False)
            nc.tensor.matmul(rz[:, NB:2 * NB],
                             whh_sb[:, k, (8 + j) * 128:(9 + j) * 128],
                             hc[:, k, :], start=(k == 0), stop=False)
            nc.tensor.matmul(hn[:],
                             whh_sb[:, k, (16 + j) * 128:(17 + j) * 128],
                             hc[:, k, :], start=(k == 0), stop=False)
        # token gather matmuls (K=9; biases folded into the table)
        nc.tensor.matmul(rz[:, 0:NB], atab_sb[:, j * 128:(j + 1) * 128],
                         tok[:], start=False, stop=True)
        nc.tensor.matmul(rz[:, NB:2 * NB],
                         atab_sb[:, (8 + j) * 128:(9 + j) * 128],
                         tok[:], start=False, stop=True)
        nc.tensor.matmul(hn[:], atab_sb[:, (16 + j) * 128:(17 + j) * 128],
                         tok[:], start=False, stop=True)
        nc.tensor.matmul(inp[:], atab_sb[:, (24 + j) * 128:(25 + j) * 128],
                         tok[:], start=True, stop=True)

        # gates
        rzs = rp.tile([128, 2 * NB], F32, tag="rzs", bufs=2,
                      name=f"rzs_{pair}_{b}_{t}_{j}")
        nc.scalar.activation(rzs[:], rz[:], AF.Sigmoid)
        t1 = rp.tile([128, NB], F32, tag="tmp", bufs=2,
                     name=f"t1_{pair}_{b}_{t}_{j}")
        nc.vector.tensor_tensor(t1[:], rzs[:, 0:NB], hn[:], op=ALU.mult)
        t2 = rp.tile([128, NB], F32, tag="tmp2", bufs=2,
                     name=f"t2_{pair}_{b}_{t}_{j}")
        nc.vector.tensor_tensor(t2[:], t1[:], inp[:], op=ALU.add)
        ng = rp.tile([128, NB], F32, tag="ng", bufs=2,
                     name=f"ng_{pair}_{b}_{t}_{j}")
        nc.scalar.activation(ng[:], t2[:], AF.Tanh)
        d = rp.tile([128, NB], F32, tag="tmp", bufs=2,
                    name=f"d_{pair}_{b}_{t}_{j}")
        nc.vector.tensor_tensor(d[:], hc[:, j, :], ng[:], op=ALU.subtract)
        e = rp.tile([128, NB], F32, tag="tmp2", bufs=2,
                    name=f"e_{pair}_{b}_{t}_{j}")
        nc.vector.tensor_tensor(e[:], rzs[:, NB:2 * NB], d[:], op=ALU.mult)
        nc.vector.tensor_tensor(hx[:, j, :], ng[:], e[:], op=ALU.add)

    # proj -> logitsT [9, NB]
    lg = pps.tile([V, NB], F32, tag="misc", bufs=2, name=f"lg_{pair}_{b}_{t}")
    for k in range(KC):
        nc.tensor.matmul(lg[:], projT_sb[:, k, :], hx[:, k, :],
                         start=(k == 0), stop=(k == KC - 1))

    # s = logitsT + (gumbel + proj_b) pretransposed; in-place into gtt
    sT = gtt
    nc.vector.tensor_tensor(sT[:], lg[:], gtt[:], op=ALU.add)

    # transpose to [128, NCH, 9]
    s128 = pps.tile([128, NCH, V], F32, tag="misc", bufs=2,
                    name=f"s128_{pair}_{b}_{t}")
    for c in range(NCH):
        nc.tensor.transpose(s128[:, c, :], sT[:, c * 128:(c + 1) * 128],
                            ident_sb[0:V, 0:V])

    # argmax -> one-hot mask (== message slice)
    m4 = rp.tile([128, NCH], F32, tag="m4", bufs=2, name=f"m4_{pair}_{b}_{t}")
    nc.vector.tensor_reduce(m4[:], s128[:, :, :], axis=mybir.AxisListType.X,
                            op=ALU.max)
    mask = rp.tile([128, NCH, V], F32, tag="mask", bufs=2,
                   name=f"mask_{pair}_{b}_{t}")
    for c in range(NCH):
        nc.vector.tensor_scalar(mask[:, c, :], s128[:, c, :],
                                m4[:, c:c + 1], None, op0=ALU.is_equal)

    nc.sync.dma_start(out=msg_v[:, cg0:cg0 + NCH, t, :], in_=mask[:])
    nc.vector.tensor_copy(eos[:, :, t], mask[:, :, EOS])

    # transpose mask back to [9, NB] for next step's token matmuls
    tokps = pps.tile([V, NB], F32, tag="misc", bufs=2,
                     name=f"tokps_{pair}_{b}_{t}")
    for c in range(NCH):
        nc.tensor.transpose(tokps[:, c * 128:(c + 1) * 128], mask[:, c, :],
                            ident_sb[:])
    nc.scalar.copy(tok[:], tokps[:])


# ---------------------------------------------------------------------------
# host side
# ---------------------------------------------------------------------------

def prep_host_inputs(x, fc_w, w_ih, w_hh, b_ih, b_hh, proj_w, proj_b, gumbel,
                     T=T_FULL):
    """Shared (core-independent) host-side tensor preparation."""
    f = np.float32
    x = np.asarray(x, f)
    B = x.shape[0]
    fc_w = np.asarray(fc_w, f)
    w_ih = np.asarray(w_ih, f)
    w_hh = np.asarray(w_hh, f)
    b_ih = np.asarray(b_ih, f)
    b_hh = np.asarray(b_hh, f)
    proj_w = np.asarray(proj_w, f)
    proj_b = np.asarray(proj_b, f)
    gumbel = np.asarray(gumbel, f)

    Hh = w_hh.shape[1]
    # x.T padded to 896 rows, k-major chunks
    xT = np.zeros((KF * 128, B), f)
    xT[:x.shape[1]] = x.T
    xT = np.ascontiguousarray(xT.reshape(KF, 128, B))
    # w_hh.T tiled [128, KC, 3H]
    whh = np.ascontiguousarray(
        w_hh.T.reshape(KC, 128, 3 * Hh).transpose(1, 0, 2))
    # token tables [9, 4H]: rz (+b_ih+b_hh) | b_hh_n replicated | in (+b_ih)
    t_rz = (w_ih[:2 * Hh] + (b_ih + b_hh)[:2 * Hh, None]).T
    t_hn = np.tile(b_hh[2 * Hh:3 * Hh][None, :], (V, 1))
    t_in = (w_ih[2 * Hh:3 * Hh] + b_ih[2 * Hh:3 * Hh, None]).T
    atab = np.ascontiguousarray(np.concatenate([t_rz, t_hn, t_in], axis=1)
                                .astype(f))
    # proj_w.T tiled [128, KC, 9]
    projT = np.ascontiguousarray(
        proj_w.T.reshape(KC, 128, V).transpose(1, 0, 2))
    # fc_w.T * 0.5, padded, k-major
    fcw = np.zeros((KF * 128, Hh), f)
    fcw[:fc_w.shape[1]] = 0.5 * fc_w.T
    fcw = np.ascontiguousarray(fcw.reshape(KF, 128, Hh))
    # gumbel + proj_b, transposed to [T, 9, B]
    gt = np.ascontiguousarray((gumbel + proj_b).transpose(0, 2, 1).astype(f))
    wt = np.ascontiguousarray(
        np.broadcast_to((T - np.arange(T)).astype(f), (128, T)))
    ident = np.eye(128, dtype=f)
    return dict(xT=xT, whh=whh, atab=atab, projT=projT, fcw=fcw, gt=gt,
                wt=wt, ident=ident)


def make_in_maps(host, BL, ncores):
    in_maps = []
    for c in range(ncores):
        sl = slice(c * BL, (c + 1) * BL)
        in_maps.append({
            "xT": np.ascontiguousarray(host["xT"][:, :, sl]),
            "whh": host["whh"],
            "atab": host["atab"],
            "projT": host["projT"],
            "fcw": host["fcw"],
            "gt": np.ascontiguousarray(host["gt"][:, :, sl]),
            "wt": host["wt"],
            "ident": host["ident"],
        })
    return in_maps


_NC_CACHE = {}


def _get_nc(BL=BL_FULL, T=T_FULL, NB=512):
    key = (BL, T, NB)
    if key not in _NC_CACHE:
        _NC_CACHE[key] = build_nc(BL, T, NB)
    return _NC_CACHE[key]


def kernel(x, tau, fc_w, w_ih, w_hh, b_ih, b_hh, proj_w, proj_b, gumbel,
           trace=False):
    from concourse.bass_utils import run_bass_kernel_spmd

    host = prep_host_inputs(x, fc_w, w_ih, w_hh, b_ih, b_hh, proj_w, proj_b,
                            gumbel)
    B = np.asarray(x).shape[0]
    BL = B // NCORES
    nc = _get_nc(BL=BL, T=np.asarray(gumbel).shape[0])
    in_maps = make_in_maps(host, BL, NCORES)
    res = run_bass_kernel_spmd(nc, in_maps, list(range(NCORES)), trace=trace)
    outs = res.results
    message = np.concatenate([o["msg"] for o in outs], axis=0)
    lengths = np.concatenate([o["lenf"] for o in outs], axis=0)
    lengths = np.rint(lengths).astype(np.int32)
    if trace:
        kernel.last_exec_time_ns = res.exec_time_ns
        kernel.last_results = res
    return message, lengths


# revision 12
# speedup vs baseline: 1.5682x; 1.5682x over previous
"""Trainium2 Bass kernel for nn_Encoder_73478300500500.

GRU-based gumbel sampler:
  h0 = gelu(x @ fc_w.T); 20 steps of GRU + proj + gumbel-argmax one-hot
  feedback; outputs (message [B,T,9] one-hot-ish fp32, lengths [B] int32).

Strategy (8 cores, pure batch data-parallel, 4096 batch rows/core):
  * All recurrent state kept transposed: hT [H=1024 partitions(8 chunks), batch free].
    Main matmul per step: ghT = w_hh @ hT done as 24 m-chunks x 8 k-chunks of
    [128,128] fp32 stationary x [128, NB] moving -> PSUM, so h_new lands in the
    layout the next step consumes (no per-step transposes of h).
  * The fed-back token is one-hot -> tok @ w_ih.T == column gather of a
    [9, 3H] table == a K=9 matmul accumulated into the same PSUM groups.
    Biases b_ih+b_hh are folded into that table (token rows sum to 1).
  * r/z gates share one [128, 2*NB] PSUM (single sigmoid per j-chunk);
    n-gate keeps h_n and i_n in separate PSUMs (n = tanh(i_n + r*h_n)).
  * proj -> logitsT [9, NB] psum; gumbel (+proj_b, host-pretransposed) added;
    PE-transpose 128-col slices to [128, 9]; free-dim reduce_max + is_equal
    against the per-partition max gives the one-hot mask == message output;
    mask transposed back to [9, NB] for the next step's token matmuls.
  * lengths: EOS column of mask accumulated per step into eos[128, NCH, T];
    post-loop: lengths = 21 - max(max_t(eos_t * (T - t)), 1).
  * fc phase per pair: fc_w prescaled by 0.5 host-side; gelu computed as
    u'*(1+erf(sqrt(2)*u')) with u' = 0.5*x@fc_w.T (stays in the
    sigmoid/tanh/erf ACT table set).
  * 2 blocks of NB=512 interleaved per "pair" to hide gate/sampling latency
    between steps; 4 sequential pairs per core.
"""

import os
import sys

import numpy as np

for _p in ("/opt/trn_rl_repo", os.path.expanduser("~/.axon_site/_ro/trn_rl_repo")):
    if os.path.isdir(_p) and _p not in sys.path:
        sys.path.insert(0, _p)

os.environ.setdefault("MYCRO_LOCAL_CACHE", "1")

import concourse.bacc as bacc  # noqa: E402
import concourse.bass as bass  # noqa: E402
import concourse.mybir as mybir  # noqa: E402
import concourse.tile as tile  # noqa: E402

F32 = mybir.dt.float32
AF = mybir.ActivationFunctionType
ALU = mybir.AluOpType

B_FULL, H, V, T_FULL = 32768, 1024, 9, 20
EOS = 8
NCORES = 8
BL_FULL = B_FULL // NCORES  # 4096
KC = H // 128               # 8 k-chunks of hidden
KF = 7                      # 784 -> 7 chunks of 128 (padded to 896)
SQRT2 = 1.4142135623730951


def build_nc(BL=BL_FULL, T=T_FULL, NB=512, reps=1):
    """Build the single-core SPMD Bass program (identical across cores).

    reps>1 re-emits the whole computation N times (identical outputs) —
    used only to measure per-invocation kernel time by wall-clock slope."""
    assert BL % (2 * NB) == 0 and NB % 128 == 0 and NB <= 512
    PAIRS = BL // (2 * NB)
    NCH = NB // 128

    nc = bacc.Bacc("TRN2", target_bir_lowering=False, debug=False)

    xT_d = nc.dram_tensor("xT", [KF, 128, BL], F32, kind="ExternalInput")
    whh_d = nc.dram_tensor("whh", [128, KC, 3 * H], F32, kind="ExternalInput")
    atab_d = nc.dram_tensor("atab", [V, 4 * H], F32, kind="ExternalInput")
    projT_d = nc.dram_tensor("projT", [128, KC, V], F32, kind="ExternalInput")
    fcw_d = nc.dram_tensor("fcw", [KF, 128, H], F32, kind="ExternalInput")
    gt_d = nc.dram_tensor("gt", [T, V, BL], F32, kind="ExternalInput")
    wt_d = nc.dram_tensor("wt", [128, T], F32, kind="ExternalInput")
    ident_d = nc.dram_tensor("ident", [128, 128], F32, kind="ExternalInput")

    msg_d = nc.dram_tensor("msg", [BL, T, V], F32, kind="ExternalOutput")
    lenf_d = nc.dram_tensor("lenf", [BL], F32, kind="ExternalOutput")

    # message viewed as [p, chunk, t, v] so a [128, NCH, 9] sbuf tile DMAs out
    # with matching element order
    msg_v = msg_d.rearrange("(c p) t v -> p c t v", p=128)

    with tile.TileContext(nc) as tc:
        with tc.tile_pool(name="persist", bufs=1) as pp:
            whh_sb = pp.tile([128, KC, 3 * H], F32, name="whh_sb")
            nc.sync.dma_start(out=whh_sb[:], in_=whh_d[:])
            atab_sb = pp.tile([V, 4 * H], F32, name="atab_sb")
            nc.sync.dma_start(out=atab_sb[:], in_=atab_d[:])
            projT_sb = pp.tile([128, KC, V], F32, name="projT_sb")
            nc.sync.dma_start(out=projT_sb[:], in_=projT_d[:])
            ident_sb = pp.tile([128, 128], F32, name="ident_sb")
            nc.sync.dma_start(out=ident_sb[:], in_=ident_d[:])
            wt_sb = pp.tile([128, T], F32, name="wt_sb")
            nc.sync.dma_start(out=wt_sb[:], in_=wt_d[:])

            for _rep in range(reps):
                for pair in range(PAIRS):
                    _emit_pair(nc, tc, pair, BL, T, NB, NCH,
                               whh_sb, atab_sb, projT_sb, ident_sb, wt_sb,
                               xT_d, fcw_d, gt_d, msg_v, lenf_d)
    nc.compile()
    return nc


def _hbuf_idx(T):
    """3-buffer rotation: A(t): a[t]->a[t+1]; B(t): b[t]->b[t+1], with
    a[t+1] = 3-a[t]-b[t], b[t+1] = a[t]. Guarantees the buffer being written
    is dead for both blocks (reads of it finished >= a full block-step ago)."""
    a, b = [0], [1]
    for t in range(T):
        a.append(3 - a[-1] - b[-1])
        b.append(a[-2])
    return a, b


def _emit_pair(nc, tc, pair, BL, T, NB, NCH,
               whh_sb, atab_sb, projT_sb, ident_sb, wt_sb,
               xT_d, fcw_d, gt_d, msg_v, lenf_d):
    with tc.tile_pool(name=f"hp{pair}", bufs=1) as hp:
        hbufs = [hp.tile([128, KC, NB], F32, name=f"h_{pair}_{i}", bufs=1)
                 for i in range(3)]
        ha, hb = _hbuf_idx(T)
        eos = [hp.tile([128, NCH, T], F32, name=f"eos_{pair}_{b}", bufs=1)
               for b in range(2)]
        tok = [hp.tile([V, NB], F32, name=f"tok_{pair}_{b}", bufs=1)
               for b in range(2)]

        # ---- fc phase: h0 = gelu(x @ fc_w.T), via prescaled-by-0.5 weights
        with tc.tile_pool(name=f"fc{pair}", bufs=1) as fcp, \
             tc.tile_pool(name=f"fcps{pair}", bufs=1, space="PSUM") as fpp:
            for b in range(2):
                n0 = pair * 2 * NB + b * NB
                fps = [fpp.tile([128, NB], F32, tag="fcps", bufs=8,
                                name=f"fps_{pair}_{b}_{m}") for m in range(KC)]
                for k in range(KF):
                    fcw_t = fcp.tile([128, H], F32, tag="fcw", bufs=2,
                                     name=f"fcw_t_{pair}_{b}_{k}")
                    nc.sync.dma_start(out=fcw_t[:], in_=fcw_d[k])
                    xt_t = fcp.tile([128, NB], F32, tag="xt", bufs=2,
                                    name=f"xt_t_{pair}_{b}_{k}")
                    nc.sync.dma_start(out=xt_t[:], in_=xT_d[k, :, n0:n0 + NB])
                    for m in range(KC):
                        nc.tensor.matmul(
                            fps[m][:], fcw_t[:, m * 128:(m + 1) * 128], xt_t[:],
                            start=(k == 0), stop=(k == KF - 1))
                h0 = hbufs[ha[0] if b == 0 else hb[0]]
                for m in range(KC):
                    e_t = fcp.tile([128, NB], F32, tag="erf", bufs=2,
                                   name=f"erf_{pair}_{b}_{m}")
                    nc.scalar.activation(e_t[:], fps[m][:], AF.Erf, scale=SQRT2)
                    # h0 = u' * (1 + erf(sqrt2*u')), u' = 0.5*x@fcwT
                    nc.vector.scalar_tensor_tensor(
                        h0[:, m, :], e_t[:], 1.0, fps[m][:],
                        op0=ALU.add, op1=ALU.mult)
                nc.vector.memset(tok[b][:, :], 0.0)
                nc.vector.memset(tok[b][0:1, :], 1.0)

        # ---- recurrence
        with tc.tile_pool(name=f"rec{pair}", bufs=1) as rp, \
             tc.tile_pool(name=f"rps{pair}", bufs=1, space="PSUM") as pps:
            for t in range(T):
                for b in range(2):
                    idx = (ha, hb)[b]
                    _emit_step(nc, tc, pair, b, t, BL, T, NB, NCH,
                               whh_sb, atab_sb, projT_sb, ident_sb,
                               hbufs[idx[t]], hbufs[idx[t + 1]],
                               eos[b], tok[b], gt_d, msg_v, rp, pps)
            # ---- lengths
            for b in range(2):
                for c in range(NCH):
                    cg = (pair * 2 * NB + b * NB) // 128 + c
                    ltmp = rp.tile([128, T], F32, tag="ltmp", bufs=2,
                                   name=f"ltmp_{pair}_{b}_{c}")
                    nc.vector.tensor_tensor(ltmp[:], eos[b][:, c, :],
                                            wt_sb[:, 0:T], op=ALU.mult)
                    lmx = rp.tile([128, 1], F32, tag="lmx", bufs=2,
                                  name=f"lmx_{pair}_{b}_{c}")
                    nc.vector.tensor_reduce(lmx[:], ltmp[:],
                                            axis=mybir.AxisListType.X,
                                            op=ALU.max)
                    lfin = rp.tile([128, 1], F32, tag="lfin", bufs=2,
                                   name=f"lfin_{pair}_{b}_{c}")
                    nc.vector.tensor_scalar(lfin[:], lmx[:], 1.0, None,
                                            op0=ALU.max)
                    nc.vector.tensor_scalar(lfin[:], lfin[:], -1.0,
                                            float(T + 1),
                                            op0=ALU.mult, op1=ALU.add)
                    nc.sync.dma_start(out=lenf_d[cg * 128:(cg + 1) * 128],
                                      in_=lfin[:])


def _emit_step(nc, tc, pair, b, t, BL, T, NB, NCH,
               whh_sb, atab_sb, projT_sb, ident_sb,
               hc, hx, eos, tok, gt_d, msg_v, rp, pps):
    """One GRU step for one block: reads hc (h_t), writes hx (h_{t+1})."""
    n0 = pair * 2 * NB + b * NB
    cg0 = n0 // 128

    # prefetch gumbel for this (block, step)
    gtt = rp.tile([V, NB], F32, tag=f"gt{b}", bufs=2, name=f"gt_{pair}_{b}_{t}")
    nc.sync.dma_start(out=gtt[:], in_=gt_d[t, :, n0:n0 + NB])

    for j in range(KC):
        rz = pps.tile([128, 2 * NB], F32, tag="rz", bufs=2,
                      name=f"rz_{pair}_{b}_{t}_{j}")
        hn = pps.tile([128, NB], F32, tag="hn", bufs=2,
                      name=f"hn_{pair}_{b}_{t}_{j}")
        inp = pps.tile([128, NB], F32, tag="misc", bufs=2,
                       name=f"inp_{pair}_{b}_{t}_{j}")
        for k in range(KC):
            nc.tensor.matmul(rz[:, 0:NB],
                             whh_sb[:, k, j * 128:(j + 1) * 128],
                             hc[:, k, :], start=(k == 0), stop=False)
            nc.tensor.matmul(rz[:, NB:2 * NB],
                             whh_sb[:, k, (8 + j) * 128:(9 + j) * 128],
                             hc[:, k, :], start=(k == 0), stop=False)
            nc.tensor.matmul(hn[:],
                             whh_sb[:, k, (16 + j) * 128:(17 + j) * 128],
                             hc[:, k, :], start=(k == 0), stop=False)
        # token gather matmuls (K=9; biases folded into the table)
        nc.tensor.matmul(rz[:, 0:NB], atab_sb[:, j * 128:(j + 1) * 128],
                         tok[:], start=False, stop=True)
        nc.tensor.matmul(rz[:, NB:2 * NB],
                         atab_sb[:, (8 + j) * 128:(9 + j) * 128],
                         tok[:], start=False, stop=True)
        nc.tensor.matmul(hn[:], atab_sb[:, (16 + j) * 128:(17 + j) * 128],
                         tok[:], start=False, stop=True)
        nc.tensor.matmul(inp[:], atab_sb[:, (24 + j) * 128:(25 + j) * 128],
                         tok[:], start=True, stop=True)

        # gates
        rzs = rp.tile([128, 2 * NB], F32, tag="rzs", bufs=2,
                      name=f"rzs_{pair}_{b}_{t}_{j}")
        nc.scalar.activation(rzs[:], rz[:], AF.Sigmoid)
        t1 = rp.tile([128, NB], F32, tag="tmp", bufs=2,
                     name=f"t1_{pair}_{b}_{t}_{j}")
        nc.vector.tensor_tensor(t1[:], rzs[:, 0:NB], hn[:], op=ALU.mult)
        t2 = rp.tile([128, NB], F32, tag="tmp2", bufs=2,
                     name=f"t2_{pair}_{b}_{t}_{j}")
        nc.vector.tensor_tensor(t2[:], t1[:], inp[:], op=ALU.add)
        ng = rp.tile([128, NB], F32, tag="ng", bufs=2,
                     name=f"ng_{pair}_{b}_{t}_{j}")
        nc.scalar.activation(ng[:], t2[:], AF.Tanh)
        d = rp.tile([128, NB], F32, tag="tmp", bufs=2,
                    name=f"d_{pair}_{b}_{t}_{j}")
        nc.vector.tensor_tensor(d[:], hc[:, j, :], ng[:], op=ALU.subtract)
        e = rp.tile([128, NB], F32, tag="tmp2", bufs=2,
                    name=f"e_{pair}_{b}_{t}_{j}")
        nc.vector.tensor_tensor(e[:], rzs[:, NB:2 * NB], d[:], op=ALU.mult)
        nc.vector.tensor_tensor(hx[:, j, :], ng[:], e[:], op=ALU.add)

    # proj -> logitsT [9, NB]
    lg = pps.tile([V, NB], F32, tag="misc", bufs=2, name=f"lg_{pair}_{b}_{t}")
    for k in range(KC):
        nc.tensor.matmul(lg[:], projT_sb[:, k, :], hx[:, k, :],
                         start=(k == 0), stop=(k == KC - 1))

    # s = logitsT + (gumbel + proj_b) pretransposed; in-place into gtt
    sT = gtt
    nc.vector.tensor_tensor(sT[:], lg[:], gtt[:], op=ALU.add)

    # transpose to [128, NCH, 9]
    s128 = pps.tile([128, NCH, V], F32, tag="misc", bufs=2,
                    name=f"s128_{pair}_{b}_{t}")
    for c in range(NCH):
        nc.tensor.transpose(s128[:, c, :], sT[:, c * 128:(c + 1) * 128],
                            ident_sb[0:V, 0:V])

    # argmax -> one-hot mask (== message slice)
    m4 = rp.tile([128, NCH], F32, tag="m4", bufs=2, name=f"m4_{pair}_{b}_{t}")
    nc.vector.tensor_reduce(m4[:], s128[:, :, :], axis=mybir.AxisListType.X,
                            op=ALU.max)
    mask = rp.tile([128, NCH, V], F32, tag="mask", bufs=2,
                   name=f"mask_{pair}_{b}_{t}")
    for c in range(NCH):
        nc.vector.tensor_scalar(mask[:, c, :], s128[:, c, :],
                                m4[:, c:c + 1], None, op0=ALU.is_equal)

    nc.sync.dma_start(out=msg_v[:, cg0:cg0 + NCH, t, :], in_=mask[:])
    nc.vector.tensor_copy(eos[:, :, t], mask[:, :, EOS])

    # transpose mask back to [9, NB] for next step's token matmuls
    tokps = pps.tile([V, NB], F32, tag="misc", bufs=2,
                     name=f"tokps_{pair}_{b}_{t}")
    for c in range(NCH):
        nc.tensor.transpose(tokps[:, c * 128:(c + 1) * 128], mask[:, c, :],
                            ident_sb[:])
    nc.scalar.copy(tok[:], tokps[:])


# ---------------------------------------------------------------------------
# host side
# ---------------------------------------------------------------------------

def prep_host_inputs(x, fc_w, w_ih, w_hh, b_ih, b_hh, proj_w, proj_b, gumbel,
                     T=T_FULL):
    """Shared (core-independent) host-side tensor preparation."""
    f = np.float32
    x = np.asarray(x, f)
    B = x.shape[0]
    fc_w = np.asarray(fc_w, f)
    w_ih = np.asarray(w_ih, f)
    w_hh = np.asarray(w_hh, f)
    b_ih = np.asarray(b_ih, f)
    b_hh = np.asarray(b_hh, f)
    proj_w = np.asarray(proj_w, f)
    proj_b = np.asarray(proj_b, f)
    gumbel = np.asarray(gumbel, f)

    Hh = w_hh.shape[1]
    # x.T padded to 896 rows, k-major chunks
    xT = np.zeros((KF * 128, B), f)
    xT[:x.shape[1]] = x.T
    xT = np.ascontiguousarray(xT.reshape(KF, 128, B))
    # w_hh.T tiled [128, KC, 3H]
    whh = np.ascontiguousarray(
        w_hh.T.reshape(KC, 128, 3 * Hh).transpose(1, 0, 2))
    # token tables [9, 4H]: rz (+b_ih+b_hh) | b_hh_n replicated | in (+b_ih)
    t_rz = (w_ih[:2 * Hh] + (b_ih + b_hh)[:2 * Hh, None]).T
    t_hn = np.tile(b_hh[2 * Hh:3 * Hh][None, :], (V, 1))
    t_in = (w_ih[2 * Hh:3 * Hh] + b_ih[2 * Hh:3 * Hh, None]).T
    atab = np.ascontiguousarray(np.concatenate([t_rz, t_hn, t_in], axis=1)
                                .astype(f))
    # proj_w.T tiled [128, KC, 9]
    projT = np.ascontiguousarray(
        proj_w.T.reshape(KC, 128, V).transpose(1, 0, 2))
    # fc_w.T * 0.5, padded, k-major
    fcw = np.zeros((KF * 128, Hh), f)
    fcw[:fc_w.shape[1]] = 0.5 * fc_w.T
    fcw = np.ascontiguousarray(fcw.reshape(KF, 128, Hh))
    # gumbel + proj_b, transposed to [T, 9, B]
    gt = np.ascontiguousarray((gumbel + proj_b).transpose(0, 2, 1).astype(f))
    wt = np.ascontiguousarray(
        np.broadcast_to((T - np.arange(T)).astype(f), (128, T)))
    ident = np.eye(128, dtype=f)
    return dict(xT=xT, whh=whh, atab=atab, projT=projT, fcw=fcw, gt=gt,
                wt=wt, ident=ident)


def make_in_maps(host, BL, ncores):
    in_maps = []
    for c in range(ncores):
        sl = slice(c * BL, (c + 1) * BL)
        in_maps.append({
            "xT": np.ascontiguousarray(host["xT"][:, :, sl]),
            "whh": host["whh"],
            "atab": host["atab"],
            "projT": host["projT"],
            "fcw": host["fcw"],
            "gt": np.ascontiguousarray(host["gt"][:, :, sl]),
            "wt": host["wt"],
            "ident": host["ident"],
        })
    return in_maps


_NC_CACHE = {}


def _get_nc(BL=BL_FULL, T=T_FULL, NB=512):
    key = (BL, T, NB)
    if key not in _NC_CACHE:
        _NC_CACHE[key] = build_nc(BL, T, NB)
    return _NC_CACHE[key]


def kernel(x, tau, fc_w, w_ih, w_hh, b_ih, b_hh, proj_w, proj_b, gumbel,
           trace=False):
    from concourse.bass_utils import run_bass_kernel_spmd

    host = prep_host_inputs(x, fc_w, w_ih, w_hh, b_ih, b_hh, proj_w, proj_b,
                            gumbel)
    B = np.asarray(x).shape[0]
    BL = B // NCORES
    nc = _get_nc(BL=BL, T=np.asarray(gumbel).shape[0])
    in_maps = make_in_maps(host, BL, NCORES)
    res = run_bass_kernel_spmd(nc, in_maps, list(range(NCORES)), trace=trace)
    outs = res.results
    message = np.concatenate([o["msg"] for o in outs], axis=0)
    lengths = np.concatenate([o["lenf"] for o in outs], axis=0)
    lengths = np.rint(lengths).astype(np.int32)
    if trace:
        kernel.last_exec_time_ns = res.exec_time_ns
        kernel.last_results = res
    return message, lengths
